# revision 38
# baseline (speedup 1.0000x reference)
"""Causal self-attention (B=4, S=2048, D=1024, H=16, HD=64) on 8 trn2 cores.

Sharding: core c handles batch b = c//2 and head-group g = c%2 (8 heads).
Each core computes its 8 heads' attention plus the partial output
projection over its d-slice; the host adds the two partial y's per batch.

Device layout is fully transposed ([feature, seq]) so every matmul
contraction lands on the partition dim with no on-device transposes:
  qkvT = wqkvT^T @ xT        (bf16 inputs, fp32 psum, e on partitions)
  scoresT[s_k, s_q] = kT^T @ qT     (bf16, causal-trimmed; a head pair
                                     runs as concurrent PE row-groups)
  pT = exp(scoresT/8)        (ACT, bf16 out; boundary blocks masked
                              post-exp on DVE -- PV lags scores by two
                              steps so the exp/mask hop never stalls PE)
  aug[128, s_q] = v_aug^T @ pT   (bf16; rows 0-63 = ones block -> 64
                                  replicated softmax denominators,
                                  reciprocal'd by the fast DVE approx)
  yT = wprojT^T @ (outT / denom)  (bf16 weights, bf16 y; host sums the
                                   two partial y's per batch in f32)
QKV(n=j+1) chains pace into attention column j; proj chains pace into
the later exp-bound columns; the last column's proj is two-stage so
only one matmul per chain sits behind the final softmax normalize.
"""

from contextlib import ExitStack

import ml_dtypes
import numpy as np

import concourse.bacc as bacc
import concourse.mybir as mybir
import concourse.tile as tile
from concourse._compat import with_exitstack
from concourse.bass import ds, ts  # noqa: E402
from concourse.bass_utils import run_bass_kernel_spmd
from concourse.masks import make_identity, make_upper_triangular

B, S, D = 4, 2048, 1024
H, HD = 16, 64
P = 128
GH = 8            # heads per core
DS = GH * HD      # 512, d-slice per core
E = 3 * DS        # 1536 qkv features per core
KD = D // P       # 8 contraction subtiles for qkv
KP = DS // P      # 4 contraction subtiles for proj
NJ = S // 512     # 4 s_q tiles of 512
NST = S // P      # 16 s_k tiles of 128
F32 = mybir.dt.float32
F32R = mybir.dt.float32r
BF16 = mybir.dt.bfloat16
EXP = mybir.ActivationFunctionType.Exp
MASKVAL = -240.0  # exp((s + MASKVAL)/8) = e^(s/8) * e^-30 ~ 1e-13


@with_exitstack
def _emit(ctx: ExitStack, tc: tile.TileContext, xT, wqkvT, wprojT, yT):
    nc = tc.nc

    xT_t = xT.rearrange("(ko ki) s -> ki ko s", ki=P)      # [128, 8, 2048]
    wq_t = wqkvT.rearrange("(ko ki) e -> ki ko e", ki=P)   # [128, 8, 1536]
    wp_t = wprojT.rearrange("(ko ki) e -> ki ko e", ki=P)  # [128, 4, 1024]
    yT_t = yT.rearrange("(mo mi) s -> mi mo s", mi=P)      # [128, 8, 2048]

    const = ctx.enter_context(tc.tile_pool(name="const", bufs=1))
    qk_pool = ctx.enter_context(tc.tile_pool(name="qkp", bufs=1))
    big = ctx.enter_context(tc.tile_pool(name="big", bufs=1))
    pt_pool = ctx.enter_context(tc.tile_pool(name="ptp", bufs=6))
    xin = ctx.enter_context(tc.tile_pool(name="xin", bufs=2))
    ot_pool = ctx.enter_context(tc.tile_pool(name="otp", bufs=3))
    sm = ctx.enter_context(tc.tile_pool(name="sm", bufs=3))
    yout = ctx.enter_context(tc.tile_pool(name="yo", bufs=3))
    ps_sc = ctx.enter_context(tc.tile_pool(name="ps_sc", bufs=2, space="PSUM"))
    ps_pv = ctx.enter_context(tc.tile_pool(name="ps_pv", bufs=2, space="PSUM"))

    xts = [None] * NJ

    def load_x(n, half=None):
        # x and q/k weights are k-split into two tiles so column-0 chains
        # start as soon as the first halves land
        if xts[n] is None:
            xts[n] = [None, None]
        for h in ((0, 1) if half is None else (half,)):
            xt = xin.tile([P, KD // 2, 512], BF16, tag=f"xt{h}", name="xt")
            nc.sync.dma_start(xt[:], xT_t[:, ds(4 * h, 4), ts(n, 512)])
            xts[n][h] = xt

    # DMA priority order: x block 0, q weights (chains m=0-3), k weights
    # (m=4-7), v weights, proj weights last (first consumed at j=1).
    # wqk as 4 separate tiles (k-row half x q/k column half): tile-level
    # dependency tracking means the first chain only waits for its own
    # quarter, not the whole weight load
    wqks = [[const.tile([P, KD // 2, DS], BF16, name=f"wqk{r}{c}")
             for c in range(2)] for r in range(2)]
    load_x(0, half=0)
    nc.sync.dma_start(wqks[0][0][:], wq_t[:, 0:4, 0:DS])
    load_x(0, half=1)
    nc.sync.dma_start(wqks[1][0][:], wq_t[:, 4:8, 0:DS])
    nc.sync.dma_start(wqks[0][1][:], wq_t[:, 0:4, DS:2 * DS])
    nc.sync.dma_start(wqks[1][1][:], wq_t[:, 4:8, DS:2 * DS])
    wv = const.tile([P, KD, DS], BF16)
    nc.sync.dma_start(wv[:], wq_t[:, :, 2 * DS:3 * DS])
    wp = const.tile([P, KP, D], BF16)
    nc.sync.dma_start(wp[:], wp_t)

    ident = const.tile([P, P], BF16)
    make_identity(nc, ident[:])
    mask = const.tile([P, P], BF16)
    make_upper_triangular(nc, mask[:], val=1.0, diag=True)

    # qkT: e-tiles 0-3 = q head pairs, 4-7 = k head pairs; [e_in, tile, s]
    qk = qk_pool.tile([P, 8, S], BF16)
    # v natural layout + 64-wide ones block per head: [s_in, s_tile, head, 128]
    # PV with this lhsT gives psum rows 0-63 = denom copies, 64-127 = out
    # (denoms at partition offset 0: reciprocal_approx_fast silently
    # returns garbage on partition-offset inputs).
    vaug = big.tile([P, NST, GH, 2 * HD], BF16)
    ones = const.tile([P, 1], F32)
    nc.vector.memset(ones[:], 1.0)
    nc.vector.tensor_copy(vaug[:, :, :, 0:HD], ones.to_broadcast((P, NST, GH, HD)))

    deferred = []  # chain copies deferred one pacer tick (see Pacer)

    def drain_deferred(k=None):
        n = len(deferred) if k is None else min(k, len(deferred))
        for _ in range(n):
            deferred.pop(0)()

    def qkv_qk_chain(n, m):
        ps = ps_sc.tile([P, 1024], F32, tag="sc", name="ps")[:, 0:512]
        for k in range(KD):
            nc.tensor.matmul(ps[:], wqks[k // 4][m // 4][:, k % 4, ts(m % 4, P)],
                             xts[n][k // 4][:, k % 4, :],
                             start=(k == 0), stop=(k == KD - 1))
        deferred.append(
            lambda: nc.vector.tensor_copy(qk[:, m, ts(n, 512)], ps[:]))

    def qkv_v_chain(n, ss):
        st = n * 4 + ss
        ps = ps_sc.tile([P, 1024], F32, tag="sc", name="ps")[:, 0:512]
        for k in range(KD):
            nc.tensor.matmul(ps[:], xts[n][k // 4][:, k % 4, ts(ss, P)],
                             wv[:, k, :],
                             start=(k == 0), stop=(k == KD - 1))
        deferred.append(lambda: nc.vector.tensor_copy(
            vaug[:, st, :, HD:], ps.rearrange("p (h d) -> p h d", h=GH)))

    outTs = [None] * NJ

    def attn_pair(l, j, pacer=None):
        outT = outTs[j]
        imax = 4 * (j + 1)
        pv = ps_pv.tile([P, 1024], F32, tag="pv", name="pv")

        def pv_step(i, pt, off):
            nc.tensor.matmul(pv[:, off:512], vaug[:, i, 2 * l, :],
                             pt[:, off:512],
                             start=(i == 0), stop=(i == imax - 1))
            nc.tensor.matmul(pv[:, 512 + off:1024], vaug[:, i, 2 * l + 1, :],
                             pt[:, 512 + off:1024],
                             start=(i == 0), stop=(i == imax - 1))

        pending = []  # PV(i) issues after scores(i+3): exp latency is
        for i in range(imax):  # three score-steps deep; the PE never waits
            if pacer is not None:
                pacer.tick()
            t = i - 4 * j  # >=0 -> diagonal boundary tile
            off = 128 * t if t > 0 else 0
            sc = ps_sc.tile([P, 1024], F32, tag="sc", name="sc")
            scv = sc.rearrange("p (u f) -> p u f", u=2)
            nc.tensor.matmul(sc[:, off:512], qk[0:64, 4 + l, ts(i, P)],
                             qk[0:64, l, ds(j * 512 + off, 512 - off)],
                             start=True, stop=True)
            nc.tensor.matmul(sc[:, 512 + off:1024], qk[64:128, 4 + l, ts(i, P)],
                             qk[64:128, l, ds(j * 512 + off, 512 - off)],
                             start=True, stop=True)
            pt = pt_pool.tile([P, 1024], BF16, tag="pt", name="pt")
            ptv = pt.rearrange("p (u f) -> p u f", u=2)
            nc.scalar.activation(ptv[:, :, off:512], scv[:, :, off:512],
                                 EXP, scale=0.125)
            if t >= 0:  # causal mask on the boundary 128-col block (DVE;
                nc.vector.tensor_tensor(  # PV runs 2 steps later, hop hidden)
                    ptv[:, :, off:off + P], ptv[:, :, off:off + P],
                    mask[:, None, :].to_broadcast((P, 2, P)),
                    mybir.AluOpType.mult)
            pending.append((i, pt, off))
            if len(pending) > 2:
                pv_step(*pending.pop(0))
        for rem in pending:
            if pacer is not None:  # keep PE fed while the last exps drain
                pacer.tick_force()
            pv_step(*rem)
        rec = sm.tile([HD, 1024], F32, tag="rec", name="rec")
        nc.vector.reciprocal_approx_fast(rec[:], pv[0:HD, :])
        for hh in (0, 1):
            nc.vector.tensor_tensor(outT[hh * HD:(hh + 1) * HD, l, :],
                                    pv[HD:2 * HD, 512 * hh:512 * (hh + 1)],
                                    rec[:, 512 * hh:512 * (hh + 1)],
                                    mybir.AluOpType.mult)
        return []

    yts = [None] * NJ

    def proj_col_chain(j, m):
        # y output is written bf16 into a per-column tile; columns j<3 go
        # out as one 2MB DMA, the final column as 4 DMAs to pipeline the
        # drain (DMA descriptor issue on Sync is ~0.8us each).
        if m == 0:
            yts[j] = yout.tile([P, 8, 512], BF16, tag="yt", name="yt")
        ps = ps_sc.tile([P, 1024], F32, tag="sc", name="ps")[:, 0:512]
        for k in range(KP):
            nc.tensor.matmul(ps[:], wp[:, k, ts(m, P)], outTs[j][:, k, :],
                             start=(k == 0), stop=(k == KP - 1))
        nc.vector.tensor_copy(yts[j][:, m, :], ps[:])
        if m == 7:
            nc.sync.dma_start(yT_t[:, :, ts(j, 512)], yts[j][:])

    # Last column's proj is two-stage so only one matmul per chain sits
    # behind the final softmax normalize: k=0..2 partials run during pair
    # 3 (staged to SBUF as bf16), the k=3 finish + drain runs after.
    stgs = [None] * 8

    def proj3_part1(m):
        ps = ps_sc.tile([P, 1024], F32, tag="sc", name="ps")[:, 0:512]
        for k in range(KP - 1):
            nc.tensor.matmul(ps[:], wp[:, k, ts(m, P)], outTs[NJ - 1][:, k, :],
                             start=(k == 0), stop=(k == KP - 2))
        stg = big.tile([P, 512], BF16, tag=f"stg{m}", name="stg")
        nc.vector.tensor_copy(stg[:], ps[:])
        stgs[m] = stg

    def proj3_stage2():
        j = NJ - 1
        yt = yout.tile([P, 8, 512], BF16, tag="yt", name="yt")
        for m in range(8):
            # alternate psum pools (ps_pv is free post-attention) so four
            # chains are in flight and the drain copies never gate the PE
            pool = ps_sc if m % 2 == 0 else ps_pv
            ps = pool.tile([P, 1024], F32, tag="sc" if m % 2 == 0 else "pv",
                           name="ps")[:, 0:512]
            nc.tensor.matmul(ps[:], ident[:], stgs[m][:],
                             start=True, stop=False)
            nc.tensor.matmul(ps[:], wp[:, KP - 1, ts(m, P)],
                             outTs[j][:, KP - 1, :], start=False, stop=True)
            if m % 2 == 0:  # split the drain copies across ACT + DVE
                nc.scalar.copy(yt[:, m, :], ps[:])
            else:
                nc.vector.tensor_copy(yt[:, m, :], ps[:])
            if m % 2 == 1:
                nc.sync.dma_start(yT_t[:, m - 1:m + 1, ts(j, 512)],
                                  yt[:, m - 1:m + 1, :])

    class Pacer:
        # Bresenham-paced emission of filler matmul chains between
        # attention iterations, to keep the PE dense (HAM stays warm).
        # Urgent thunks (deferred softmax normalizes) fire one per tick
        # ahead of the paced stream so DVE recips interleave with, not
        # ahead of, the next pair's mask multiplies.
        def __init__(self, thunks, total_ticks):
            self.thunks = list(thunks)
            self.total = max(1, total_ticks)
            self.ticks = 0
            self.fired = 0
            self.urgent = []

        def inject(self, thunks):
            self.urgent.extend(thunks)

        def tick(self):
            self.ticks += 1
            drain_deferred(1)
            if self.urgent:
                self.urgent.pop(0)()
                return
            while (self.fired < len(self.thunks)
                   and self.fired * self.total < self.ticks * len(self.thunks)):
                self.thunks[self.fired]()
                self.fired += 1

        def tick_force(self):
            # pair-drain tick: always burn a thunk if one remains, so the
            # PE has work while the final exps of the pair complete
            self.ticks += 1
            drain_deferred(1)
            if self.urgent:
                self.urgent.pop(0)()
                return
            if self.fired < len(self.thunks):
                self.thunks[self.fired]()
                self.fired += 1

        def flush(self):
            for t in self.urgent:
                t()
            self.urgent = []
            while self.fired < len(self.thunks):
                self.thunks[self.fired]()
                self.fired += 1
            drain_deferred()

    # prelude: QKV for the first s-block; copies drain one chain behind
    for m in range(8):
        qkv_qk_chain(0, m)
        drain_deferred(1) if m else None
    for ss in range(4):
        qkv_v_chain(0, ss)
        drain_deferred(1)
    drain_deferred()

    # attention column j; QKV(j+1) and proj(j-1) chains paced into the
    # attention iteration stream
    for j in range(NJ):
        outTs[j] = ot_pool.tile([P, KP, 512], BF16, tag="outT", name="outT")
        if j + 1 < NJ:
            load_x(j + 1)
        thunks = []
        if j + 1 < NJ:
            for m in range(8):
                thunks.append(lambda n=j + 1, m=m: qkv_qk_chain(n, m))
            for ss in range(4):
                thunks.append(lambda n=j + 1, ss=ss: qkv_v_chain(n, ss))
        # proj filler goes to the latest (ACT-bound) columns: proj(0)
        # during column 2, proj(1)+proj(2) during column 3's pairs 0-2.
        projs = [j - 2] if j >= 2 else []
        if j == NJ - 1:
            projs.append(j - 1)
        for jj in projs:
            for m in range(8):
                thunks.append(lambda jj=jj, m=m: proj_col_chain(jj, m))
        tpp = 4 * (j + 1) + 2  # ticks per pair; total is overstated ~25%
        # so Bresenham holds thunks in reserve for the forced drain ticks
        if j < NJ - 1:
            pacer = Pacer(thunks, 5 * tpp)
            for l in range(4):
                pacer.inject(attn_pair(l, j, pacer))
            pacer.flush()
        else:  # pairs 0-2 pace proj(j-1); pair 3 paces proj3 stage 1
            pacer = Pacer(thunks, 4 * tpp)
            for l in range(3):
                pacer.inject(attn_pair(l, j, pacer))
            pacer.flush()
            pacer = Pacer([lambda m=m: proj3_part1(m) for m in range(8)],
                          tpp + 4)
            pacer.inject(attn_pair(3, j, pacer))
            pacer.flush()
    proj3_stage2()


_NC = None


def build_nc():
    global _NC
    if _NC is not None:
        return _NC
    nc = bacc.Bacc("TRN2", target_bir_lowering=False, debug=False)
    xT = nc.dram_tensor("xT", [D, S], BF16, kind="ExternalInput")
    wqkvT = nc.dram_tensor("wqkvT", [D, E], BF16, kind="ExternalInput")
    wprojT = nc.dram_tensor("wprojT", [DS, D], BF16, kind="ExternalInput")
    yT = nc.dram_tensor("yT", [D, S], BF16, kind="ExternalOutput")
    with tile.TileContext(nc) as tc:
        _emit(tc, xT.ap(), wqkvT.ap(), wprojT.ap(), yT.ap())
    nc.compile()
    _NC = nc
    return nc


def make_in_maps(x, w_attn, w_proj):
    x = np.ascontiguousarray(np.asarray(x, dtype=np.float32))
    w_attn = np.asarray(w_attn, dtype=np.float32)
    w_proj = np.asarray(w_proj, dtype=np.float32)
    in_maps = []
    for c in range(8):
        b, g = divmod(c, 2)
        rows = slice(g * DS, (g + 1) * DS)
        wqkv_c = np.concatenate(
            [w_attn[0 * D:1 * D][rows], w_attn[1 * D:2 * D][rows],
             w_attn[2 * D:3 * D][rows]], axis=0)          # [1536, 1024]
        in_maps.append({
            "xT": np.ascontiguousarray(x[b].T).astype(ml_dtypes.bfloat16),
            "wqkvT": np.ascontiguousarray(wqkv_c.T).astype(ml_dtypes.bfloat16),
            "wprojT": np.ascontiguousarray(w_proj[:, rows].T).astype(ml_dtypes.bfloat16),
        })
    return in_maps


def gather(results):
    y = np.empty((B, S, D), dtype=np.float32)
    for b in range(B):
        yT = (results[2 * b]["yT"].astype(np.float32)
              + results[2 * b + 1]["yT"].astype(np.float32))
        y[b] = yT.T
    return y


def run(x, w_attn, w_proj, trace=False, tmpdir=None):
    nc = build_nc()
    in_maps = make_in_maps(x, w_attn, w_proj)
    res = run_bass_kernel_spmd(nc, in_maps, list(range(8)),
                               trace=trace, tmpdir=tmpdir)
    return gather(res.results), res


def kernel(x, w_attn, w_proj):
    y, _ = run(x, w_attn, w_proj)
    return y
